# revision 5
# baseline (speedup 1.0000x reference)
"""Trainium2 Bass kernel for CausalWanSelfAttention (KV-cache-bias attention).

Math: disjoint-segment attention + LSE merge == global softmax with a
per-key bias b_l (log 0.1 on keys in [frame_seqlen, current_block_start)).
exp needs no max-subtraction; out = (E @ V) / (1^T E) with
E = exp(scale*S + b_l), bias folded into exp per partition (= key).

Sharding: 24 units = (head, q-half), 3 per core; unit = 1024 queries x
8192 keys in 64 key-chunks of 128.

v10: chunk-granular pipeline with exp split across ACT *and* DVE.
Q is host-prescaled by 128*scale/ln2, so S' = 128*log2-units. Work unit
= one key chunk c: S' tile [128k, 1024q] (2 PSUM banks, s_pool bufs=3).
ACT chunks: one fused exp instr (scale=ln2/128, per-key AP bias -- works
for boundary-straddling chunks too). DVE chunks (every DVE_EVERY-th
slot, uniform-bias only): one custom-DVE op EXP2_BITS_ANT that computes
bf16-bit-pattern exp directly: U0 = S'+C0; magic-round frac g =
U0-((U0+M)-M); out = int16(U0 + a*g^2 + C1), bitcast bf16. Quadratic
2^frac correction => max rel err 0.47% (validated bit-exact on HW).
This offloads ~1/DVE_EVERY of the exp from the ACT critical path (~188us
busy in v9) onto the DVE, pushing the kernel toward the PE roofline
(~157us: 192 chunks/core x ~815ns for ldw+4 MM N=512, measured).
B lags 2 slots; norm pair/quad adds now span both q-halves in one
[128,1024] DVE op. Host does the final 128-partition norm sum + divide.
"""

import math
import sys

for _p in ("/opt/trn_rl_repo",):
    if _p not in sys.path:
        sys.path.insert(0, _p)

import numpy as np
import ml_dtypes

import concourse.bass as bass
import concourse.mybir as mybir
import concourse.tile as tile
from concourse import bacc
from concourse.bass_utils import run_bass_kernel_spmd

BF16 = mybir.dt.bfloat16
F32 = mybir.dt.float32
I16 = mybir.dt.int16
NP_BF16 = ml_dtypes.bfloat16

B, LQ, LK, H, D = 1, 2048, 8192, 12, 128
N_CORES = 8
UNITS_PER_CORE = 3          # 24 units = 12 heads x 2 q-halves
QSPAN = 1024                # queries per unit
HS = 512
NLC = LK // 128             # 64 key chunks of 128
NCT = UNITS_PER_CORE * NLC  # 192 chunk-slots per core
NQUAD = NLC // 4            # 16 norm slots (last = 4 raw chunks)
SCALE = 1.0 / math.sqrt(D)
LN2 = math.log(2.0)
LOG_BIAS = math.log(0.1)
QPRE = 128.0 * SCALE / LN2          # host pre-scale on q
ACT_SCALE = LN2 / 128.0             # ACT exp free-affine compensation

# custom DVE exp constants (see exp_dve_model.py; HW-validated)
PHI = -1.2
A_COEF = 0.00257
C_VERT = -10.775
MAGIC = float(np.float32(1.5 * 2 ** 30))
DVE_S0_BASE = 16256.0 + 64.0 + PHI
DVE_S1 = C_VERT - 64.0 - PHI

import os
DVE_EVERY = int(os.environ.get("KDVE", "4"))
DVE_PHASE = 1
LAG = int(os.environ.get("KLAG", "2"))
SBUFS = int(os.environ.get("KSBUFS", "3"))

_CACHED = {}
TIME_LOOP = 1     # timing experiments only: hardware-loop the body N times

_EXP2_OP = None


def _register_exp2_op():
    global _EXP2_OP
    if _EXP2_OP is not None:
        return _EXP2_OP
    from concourse import dve_ops
    from concourse.dve_spec import (Spec, Src0, C0, C1, C2, C3,
                                    _spill_c3_to_src1, lower)
    from concourse.dve_uop import DveOpSpec
    for op in dve_ops.OPS:
        if op.name == "EXP2_BITS_ANT":
            _EXP2_OP = op
            return op
    U0 = Src0 + C0
    t = U0 + C2
    x = t - C2
    g = U0 - x
    w = g * g
    wa = w * C3
    body = _spill_c3_to_src1((U0 + wa) + C1)

    def ref(in0, in1, s0, s1, imm2):
        f32 = np.float32
        a = np.asarray(in1, f32).reshape(-1, 1)
        U0 = (in0.astype(f32) + f32(s0)).astype(f32)
        t = (U0 + f32(imm2)).astype(f32)
        x = (t - f32(imm2)).astype(f32)
        g = (U0 - x).astype(f32)
        wa = ((g * g).astype(f32) * a).astype(f32)
        return ((U0 + wa).astype(f32) + f32(s1)).astype(f32)

    spec = Spec(body=body, reference=ref)
    row = max(dve_ops._SUB_OPCODE_FOR_NAME.values()) + 1
    assert row < 0x20
    shas = {}
    for ver in ("v3", "v4"):
        try:
            uops = lower(spec, ver=ver)
            shas[ver] = DveOpSpec(name="EXP2_BITS_ANT", opcode=row, uops=uops,
                                  rd1_en=True).sha(ver)
        except Exception:
            pass
    op = dve_ops.DveOp("EXP2_BITS_ANT", spec, subdim=False, uops_sha=shas)
    dve_ops.OPS.append(op)
    dve_ops.CUSTOM_DVE_SPECS[op.name] = spec
    dve_ops._SUB_OPCODE_FOR_NAME[op.name] = row
    _EXP2_OP = op
    return op


def _chunk_classes(fs, bs):
    """Per key-chunk bias class: '0' uniform zero, 'b' uniform LOG_BIAS,
    'm' mixed (segment boundary strictly inside the chunk)."""
    out = []
    for c in range(NLC):
        lo, hi = c * 128, (c + 1) * 128
        if (lo < fs < hi) or (lo < bs < hi):
            out.append('m')
        elif fs <= lo < bs:
            out.append('b')
        else:
            out.append('0')
    return tuple(out)


def _build_program(classes):
    exp_op = _register_exp2_op()
    nc = bacc.Bacc("TRN2", target_bir_lowering=False, debug=False,
                   enable_asserts=False)

    qt_d = nc.dram_tensor("qt", [UNITS_PER_CORE, 128, QSPAN], BF16,
                          kind="ExternalInput")
    kt_d = nc.dram_tensor("kt", [UNITS_PER_CORE, 128, LK], BF16,
                          kind="ExternalInput")
    vl_d = nc.dram_tensor("vl", [UNITS_PER_CORE, LK, 128], BF16,
                          kind="ExternalInput")
    bias_d = nc.dram_tensor("bias", [128, NLC], F32, kind="ExternalInput")
    ot_d = nc.dram_tensor("ot", [UNITS_PER_CORE, 128, QSPAN], F32,
                          kind="ExternalOutput")
    no_d = nc.dram_tensor("no", [UNITS_PER_CORE, NQUAD + 3, 128, QSPAN], BF16,
                          kind="ExternalOutput")

    qt_ap = qt_d.ap()
    kt_ap = kt_d.ap()
    vl_ap = vl_d.ap().rearrange("u (c p) d -> u p c d", p=128)
    bias_ap = bias_d.ap()
    ot_ap = ot_d.ap()
    no_ap = no_d.ap()

    def is_dve(g):
        return (g % DVE_EVERY == DVE_PHASE) and classes[g % NLC] != 'm'

    with tile.TileContext(nc) as tc:
        with (
            tc.tile_pool(name="kt_pool", bufs=2) as kt_pool,
            tc.tile_pool(name="vl_pool", bufs=2) as vl_pool,
            tc.tile_pool(name="qt_pool", bufs=2) as qt_pool,
            tc.tile_pool(name="cn_pool", bufs=1) as cn_pool,
            tc.tile_pool(name="e_pool", bufs=6) as e_pool,
            tc.tile_pool(name="pp_pool", bufs=3) as pp_pool,
            tc.tile_pool(name="qq_pool", bufs=3) as qq_pool,
            tc.tile_pool(name="ob_pool", bufs=2) as ob_pool,
            tc.tile_pool(name="s_pool", bufs=SBUFS, space="PSUM") as s_pool,
            tc.tile_pool(name="o_pool", bufs=1, space="PSUM") as o_pool,
        ):
            bias_t = cn_pool.tile([128, NLC], F32, name="bias_t")
            acoef_t = cn_pool.tile([128, 1], F32, name="acoef_t")
            nc.vector.memset(acoef_t[:], A_COEF)
            # Warmup: exp table-set load overlaps the first input DMA
            warm_t = cn_pool.tile([128, 1], F32, name="warm_t")
            nc.vector.memset(warm_t[:], 0.0)
            nc.scalar.activation(warm_t[:], warm_t[:],
                                 mybir.ActivationFunctionType.Exp)

            loaded, cur = {}, {}

            def start_load(u):
                qt = qt_pool.tile([128, QSPAN], BF16, name=f"qt_u{u}",
                                  tag="qt")
                nc.sync.dma_start(out=qt[:], in_=qt_ap[u])
                kt = kt_pool.tile([128, LK], BF16, name=f"kt_u{u}", tag="kt")
                vl = vl_pool.tile([128, NLC, 128], BF16,
                                  name=f"vl_u{u}", tag="vl")
                loaded[u] = (kt, vl, qt)

            def load_slice(u, idx, den):
                kt, vl, qt = loaded.get(u) or cur[u]
                slk = bass.ts(idx, LK // den)
                nc.sync.dma_start(out=kt[:, slk], in_=kt_ap[u][:, slk])
                slv = bass.ts(idx, NLC // den)
                nc.sync.dma_start(out=vl[:, slv, :], in_=vl_ap[u][:, slv, :])

            # unit 0 lead-in: first chunks' deps first, then the rest
            qt0 = qt_pool.tile([128, QSPAN], BF16, name="qt_u0", tag="qt")
            kt0 = kt_pool.tile([128, LK], BF16, name="kt_u0", tag="kt")
            vl0 = vl_pool.tile([128, NLC, 128], BF16, name="vl_u0", tag="vl")
            loaded[0] = (kt0, vl0, qt0)
            nc.sync.dma_start(out=kt0[:, 0:256], in_=kt_ap[0][:, 0:256])
            nc.sync.dma_start(out=qt0[:], in_=qt_ap[0])
            nc.sync.dma_start(out=bias_t[:], in_=bias_ap)
            nc.sync.dma_start(out=vl0[:, 0:4, :], in_=vl_ap[0][:, 0:4, :])
            nc.sync.dma_start(out=kt0[:, 256:512], in_=kt_ap[0][:, 256:512])
            for idx in range(1, 16):
                slk = bass.ts(idx, LK // 16)
                nc.sync.dma_start(out=kt0[:, slk], in_=kt_ap[0][:, slk])
                slv = bass.ts(idx, NLC // 16)
                nc.sync.dma_start(out=vl0[:, slv, :], in_=vl_ap[0][:, slv, :])

            import contextlib
            loop_cm = (tc.For_i(0, TIME_LOOP, 1) if TIME_LOOP > 1
                       else contextlib.nullcontext())
            loop_cm.__enter__()

            ot_t = {}
            echunk = {}         # (unit, chunk) -> e tile
            pt = {}             # (unit, pair) -> pp tile
            for g in range(NCT + LAG + 1):
                if g < NCT:
                    ug, c = g // NLC, g % NLC
                    if c == 0:
                        cur[ug] = loaded.pop(ug)
                    kt, vl, qt = cur[ug]
                    sg = s_pool.tile([128, QSPAN], F32, tag="s", name=f"s_{g}")
                    for qh in range(2):
                        nc.tensor.matmul(
                            sg[:, bass.ts(qh, HS)],
                            lhsT=kt[:, bass.ts(c, 128)],
                            rhs=qt[:, bass.ts(qh, HS)],
                            start=True, stop=True)
                    e = e_pool.tile([128, QSPAN], BF16, tag="e", name=f"e_{g}")
                    if is_dve(g):
                        s0 = DVE_S0_BASE + (
                            128.0 * LOG_BIAS / LN2 if classes[c] == 'b'
                            else 0.0)
                        nc.vector._custom_dve(
                            exp_op, out=e[:].bitcast(I16), in0=sg[:],
                            in1=acoef_t[:], s0=s0, s1=DVE_S1, imm2=MAGIC)
                    else:
                        nc.scalar.activation(
                            e[:], sg[:],
                            mybir.ActivationFunctionType.Exp,
                            bias=bias_t[:, c:c + 1],
                            scale=ACT_SCALE)
                    echunk[(ug, c)] = e
                    # next unit's inputs, spread (one eighth per 4 slots)
                    if ug + 1 < UNITS_PER_CORE:
                        if c == 12:
                            start_load(ug + 1)
                        elif c >= 16 and c < 48 and c % 4 == 0:
                            load_slice(ug + 1, c // 4 - 4, 8)
                d = g - LAG
                if 0 <= d < NCT:
                    ud, c = d // NLC, d % NLC
                    if c == 0:
                        ot_t[ud] = o_pool.tile([128, QSPAN], F32,
                                               name=f"ot_u{ud}", tag="ot")
                    e = echunk[(ud, c)]
                    for qh in range(2):
                        nc.tensor.matmul(
                            ot_t[ud][:, bass.ts(qh, HS)],
                            lhsT=cur[ud][1][:, c, :],
                            rhs=e[:, bass.ts(qh, HS)],
                            start=(c == 0), stop=(c == NLC - 1))
                    if c >= NLC - 4:
                        # unit tail: raw E chunks, slots 15..18
                        nc.sync.dma_start(
                            out=no_ap[ud][NQUAD - 1 + c - (NLC - 4)],
                            in_=e[:])
                    elif c % 2 == 1:
                        pp = pp_pool.tile([128, QSPAN], BF16, tag="pp",
                                          name=f"pp_{d}")
                        nc.vector.tensor_add(
                            pp[:], echunk[(ud, c - 1)][:], e[:])
                        pt[(ud, c // 2)] = pp
                        if c % 4 == 3:
                            qq = qq_pool.tile([128, QSPAN], BF16, tag="qq",
                                              name=f"qq_{d}")
                            nc.vector.tensor_add(
                                qq[:], pt.pop((ud, c // 2 - 1))[:],
                                pt.pop((ud, c // 2))[:])
                            nc.sync.dma_start(out=no_ap[ud][c // 4], in_=qq[:])
                    if c == NLC - 1:
                        for cc in range(NLC):
                            echunk.pop((ud, cc), None)
                        ot = ot_t.pop(ud)
                        ot_sb = ob_pool.tile([128, QSPAN], F32,
                                             name=f"otsb_u{ud}", tag="otsb")
                        last = ud == UNITS_PER_CORE - 1
                        for half in range(2):
                            sl = bass.ts(half, HS)
                            if last:
                                nc.scalar.copy(ot_sb[:, sl], ot[:, sl])
                                nc.scalar.dma_start(out=ot_ap[ud][:, sl],
                                                    in_=ot_sb[:, sl])
                            else:
                                nc.vector.tensor_scalar_add(
                                    ot_sb[:, sl], ot[:, sl], 0.0)
                        if not last:
                            nc.sync.dma_start(out=ot_ap[ud], in_=ot_sb[:])

            loop_cm.__exit__(None, None, None)

    nc.compile()
    return nc


def _get_program(classes=None):
    if classes is None:
        classes = _chunk_classes(1536, 6144)
    if classes not in _CACHED:
        _CACHED[classes] = _build_program(classes)
    return _CACHED[classes]


def _host_prep(q, k, v, frame_seqlen, current_block_start):
    fs = max(0, min(int(frame_seqlen), LK))
    bs = max(0, min(int(current_block_start), LK))
    logw = np.zeros(LK, np.float32)
    logw[fs:bs] = LOG_BIAS
    bias = np.ascontiguousarray(logw.reshape(NLC, 128).T)  # [128, NLC]

    q = np.asarray(q, dtype=np.float32)
    k = np.asarray(k, dtype=np.float32)
    v = np.asarray(v, dtype=np.float32)

    qT = np.ascontiguousarray(
        (q[0] * QPRE).transpose(1, 2, 0)).astype(NP_BF16)   # [H,128,LQ]
    kT = np.ascontiguousarray(k[0].transpose(1, 2, 0)).astype(NP_BF16)
    vL = np.ascontiguousarray(v[0].transpose(1, 0, 2)).astype(NP_BF16)

    in_maps = []
    for i in range(N_CORES):
        units = [3 * i + uu for uu in range(UNITS_PER_CORE)]
        heads = [g // 2 for g in units]
        qhs = [g % 2 for g in units]
        in_maps.append({
            "qt": np.ascontiguousarray(
                np.stack([qT[h, :, qh * QSPAN:(qh + 1) * QSPAN]
                          for h, qh in zip(heads, qhs)])),
            "kt": np.ascontiguousarray(np.stack([kT[h] for h in heads])),
            "vl": np.ascontiguousarray(np.stack([vL[h] for h in heads])),
            "bias": bias,
        })
    return in_maps, _chunk_classes(fs, bs)


def _assemble(results):
    out = np.empty((B, LQ, H, D), np.float32)
    for i in range(N_CORES):
        ot = results[i]["ot"]   # [3, 128, 1024] unnormalized O^T
        nm = results[i]["no"].astype(np.float32).sum(axis=(1, 2))  # [3, 1024]
        for uu in range(UNITS_PER_CORE):
            g = 3 * i + uu
            h, qh = g // 2, g % 2
            out[0, qh * QSPAN:(qh + 1) * QSPAN, h, :] = (
                ot[uu] / nm[uu][None, :]).T
    return out


def kernel(q, k, v, frame_seqlen, current_block_start):
    in_maps, classes = _host_prep(q, k, v, frame_seqlen,
                                  current_block_start)
    nc = _get_program(classes)
    res = run_bass_kernel_spmd(nc, in_maps, core_ids=list(range(N_CORES)))
    return _assemble(res.results)
